# revision 31
# baseline (speedup 1.0000x reference)
"""Trainium2 Bass kernel for nn_MultiHeadAttention (B=4, S=2048, D=512, H=8, DH=64).

Sharding: 8 cores = 4 batches x 2 head-groups (tensor parallel over heads).
Each core projects Q/K/V for its 4 heads over the full 2048 rows (no
duplicated projection work), runs attention for those heads, and computes a
partial output projection (contraction over its 256 z-features). The host
sums the two partial outputs per batch.

Per-core pipeline (bf16 datapath, fp32 PSUM accumulation), organized as 128
uniform slots = 8 groups (4 q-blocks x 2 head-pairs) x 16 key-blocks:
  slot: S^T[k,1024] = [KA^T|KB^T](stationary) @ Q^T  (two 512-col matmuls,
        heads A/B side by side in one 2-bank PSUM tile, double-buffered)
        -> one ScalarE exp per slot ([128,1024] PSUM -> SBUF bf16)
        -> z^T[65,512] += [V_h|1](stationary) @ P^T   (lagged 4 slots)
  ScalarE does nothing but exp (the binding engine, ~131us); projections ride
  the PE slack as 512-col chunks through 2 spare PSUM banks; all bias adds,
  norm copies/mults on DVE; partition broadcasts on GpSimd.
"""

import os
import sys

import numpy as np

sys.path.insert(0, "/opt/trn_rl_repo")

import ml_dtypes
import concourse.bacc as bacc
import concourse.bass as bass
import concourse.mybir as mybir
import concourse.tile as tile
from concourse import bass_utils

F32 = mybir.dt.float32
BF16 = mybir.dt.bfloat16

B, S, D, H, DH = 4, 2048, 512, 8, 64
HG = H // 2          # heads per core (head-group)
DG = HG * DH         # 256 features per core
NKB = S // 128       # 16 k-blocks
NMC = D // 128       # 4 input-feature chunks
NQH = S // 512       # 4 query blocks of 512
NPAIR = HG // 2      # 2 head pairs per core
N_CORES = 8

Exp = mybir.ActivationFunctionType.Exp


def build_program(dbg=False):
    nc = bacc.Bacc("TRN2", target_bir_lowering=False, debug=False)
    dbg_out = {}
    if dbg:
        for nm, shp, dt in [("KT0", [128, S], BF16), ("QT0", [128, S], BF16),
                            ("VA0", [128, 260], BF16), ("P000", [128, 1024], BF16),
                            ("ZT0", [128, S], BF16), ("ZT1", [128, S], BF16),
                            ("ZC000", [65, 512], F32), ("RC000", [1, 512], F32),
                            ("ZC001", [65, 512], F32), ("RC001", [1, 512], F32)]:
            dbg_out[nm] = nc.dram_tensor(nm, shp, dt, kind="ExternalOutput").ap()

    # feature-major X^T and weights, pre-tiled on the host into the exact
    # SBUF tile layout [128, chunks*cols] (layout prep only): plain 128-row
    # DMAs with 16KB contiguous descriptors instead of 512x4KB
    xq = nc.dram_tensor("XQT", [128, NMC * S], BF16, kind="ExternalInput").ap()
    xk = nc.dram_tensor("XKT", [128, NMC * S], BF16, kind="ExternalInput").ap()
    xv = nc.dram_tensor("XVT", [128, NMC * S], BF16, kind="ExternalInput").ap()
    wq = nc.dram_tensor("Wq", [128, NMC * DG], BF16, kind="ExternalInput").ap()
    wk = nc.dram_tensor("Wk", [128, NMC * DG], BF16, kind="ExternalInput").ap()
    wv = nc.dram_tensor("Wv", [128, NMC * DG], BF16, kind="ExternalInput").ap()
    wo = nc.dram_tensor("Wo", [128, NPAIR * D], BF16, kind="ExternalInput").ap()
    bq = nc.dram_tensor("bq", [DG, 1], F32, kind="ExternalInput").ap()
    bk = nc.dram_tensor("bk", [DG, 1], F32, kind="ExternalInput").ap()
    bv = nc.dram_tensor("bv", [1, DG], F32, kind="ExternalInput").ap()
    bo = nc.dram_tensor("bo", [1, D], F32, kind="ExternalInput").ap()
    out = nc.dram_tensor("OUT", [S, D], F32, kind="ExternalOutput").ap()

    from contextlib import ExitStack

    with tile.TileContext(nc) as tc, ExitStack() as ctx:
        const = ctx.enter_context(tc.tile_pool(name="const", bufs=1))
        xt_pool = ctx.enter_context(tc.tile_pool(name="xt", bufs=1))
        w_pool = ctx.enter_context(tc.tile_pool(name="w", bufs=1))
        kt_pool = ctx.enter_context(tc.tile_pool(name="kt", bufs=1))
        qt_pool = ctx.enter_context(tc.tile_pool(name="qt", bufs=1))
        v_pool = ctx.enter_context(tc.tile_pool(name="v", bufs=1))
        p_pool = ctx.enter_context(tc.tile_pool(name="p", bufs=6))
        zt_pool = ctx.enter_context(tc.tile_pool(name="zt", bufs=1))
        nrm_pool = ctx.enter_context(tc.tile_pool(name="nrm", bufs=3))
        out_pool = ctx.enter_context(tc.tile_pool(name="outp", bufs=2))

        # PSUM: s0/s1 [128,1024] (2 banks each), zA/zB [65->128,512] (1 bank
        # each), pr0/pr1 [128,512] (1 bank each) = 8 banks exactly.
        ps = ctx.enter_context(tc.tile_pool(name="ps", bufs=1, space="PSUM"))
        pr_ctr = [0]

        def pr_tile(cols, name):
            tag = f"pr{pr_ctr[0] % 2}"
            pr_ctr[0] += 1
            return ps.tile([128, cols], F32, tag=tag, name=name,
                           padded_shape=[128, 512])

        # ---- warm the ScalarE Exp table immediately ----
        warm = nrm_pool.tile([1, 8], F32, tag="warm")
        nc.gpsimd.memset(warm[:], 0.0)
        warm2 = nrm_pool.tile([1, 8], F32, tag="warm2")
        nc.scalar.activation(warm2[:], warm[:], Exp, scale=0.125)

        # ---- weight loads: one plain DMA per tensor (host pre-tiled) ----
        def load_w(wdram, name, cols, nch=NMC):
            big = w_pool.tile([128, nch * cols], BF16, tag=f"w{name}",
                              name=f"w{name}")
            nc.sync.dma_start(big[:], wdram)
            return [big[:, cols * mc:cols * (mc + 1)] for mc in range(nch)]

        # ---- X^T loads: one plain DMA per tensor (host pre-tiled) ----
        def load_xt(xdram, name):
            big = xt_pool.tile([128, NMC * S], BF16, tag=f"xt{name}",
                               name=f"xt{name}")
            nc.sync.dma_start(big[:], xdram)
            return [big[:, S * mc:S * (mc + 1)] for mc in range(NMC)]

        # tiny bias DMAs first so the first bias-adds are never blocked
        bq_all = const.tile([128, NPAIR], F32, tag="bqa")
        nc.sync.dma_start(
            bq_all[:].rearrange("p (g o) -> p g o", g=NPAIR),
            bq.rearrange("(g p) o -> p g o", p=128),
        )
        bk_all = const.tile([128, NPAIR], F32, tag="bka")
        nc.sync.dma_start(
            bk_all[:].rearrange("p (g o) -> p g o", g=NPAIR),
            bk.rearrange("(g p) o -> p g o", p=128),
        )
        bv_row = const.tile([1, DG], F32, tag="bvr")
        nc.sync.dma_start(bv_row[:], bv[:])
        bo_row = const.tile([1, D], F32, tag="bor")
        nc.sync.dma_start(bo_row[:], bo[:])

        # all input DMAs on the sync HWDGE queue, K before Q before V so the
        # pipeline's first consumers unblock earliest
        wk_t = load_w(wk, "k", DG)
        xkt = load_xt(xk, "k")
        wq_t = load_w(wq, "q", DG)
        xqt = load_xt(xq, "q")
        wv_t = load_w(wv, "v", DG)
        xvt = load_xt(xv, "v")
        wo_t = load_w(wo, "o", D, nch=NPAIR)

        bv_bc = const.tile([128, DG], F32, tag="bvb")
        nc.gpsimd.partition_broadcast(bv_bc[:], bv_row[:], channels=128)
        bo_bc = const.tile([128, D], F32, tag="bob")
        nc.gpsimd.partition_broadcast(bo_bc[:], bo_row[:], channels=128)

        # ---- persistent SBUF results ----
        k_t = [kt_pool.tile([128, S], BF16, tag=f"kt{p}", name=f"kt{p}")
               for p in range(NPAIR)]
        q_t = [qt_pool.tile([128, S], BF16, tag=f"qt{p}", name=f"qt{p}")
               for p in range(NPAIR)]
        VW = HG * (DH + 1)  # 260: per head 64 value cols + 1 ones col
        v_aug = [v_pool.tile([128, VW], BF16, tag=f"v{kb}", name=f"v{kb}")
                 for kb in range(NKB)]
        z_t = [zt_pool.tile([128, S], BF16, tag=f"zt{p}", name=f"zt{p}")
               for p in range(NPAIR)]

        # ---- projection chunks (feeder work) ----
        def kq_chunk(dst, w_ts, b_all, pair, cb):
            pj = pr_tile(512, f"pj{cb}")
            for mc in range(NMC):
                nc.tensor.matmul(
                    pj[:],
                    w_ts[mc][:, 128 * pair:128 * (pair + 1)],
                    (xkt if dst is k_t else xqt)[mc][:, 512 * cb:512 * (cb + 1)],
                    start=(mc == 0),
                    stop=(mc == NMC - 1),
                )
            nc.vector.tensor_scalar_add(
                dst[pair][:, 512 * cb:512 * (cb + 1)], pj[:],
                b_all[:, pair:pair + 1],
            )

        def v_chunk(kb):
            nc.gpsimd.memset(
                v_aug[kb][:].rearrange("p (h c) -> p h c", h=HG)[:, :, DH:DH + 1],
                1.0,
            )
            pj = pr_tile(DG, f"pjv{kb}")
            for mc in range(NMC):
                nc.tensor.matmul(
                    pj[:],
                    xvt[mc][:, 128 * kb:128 * (kb + 1)],
                    wv_t[mc][:],
                    start=(mc == 0),
                    stop=(mc == NMC - 1),
                )
            nc.vector.tensor_add(
                v_aug[kb][:].rearrange("p (h c) -> p h c", h=HG)[:, :, 0:DH],
                pj[:].rearrange("p (h c) -> p h c", h=HG),
                bv_bc[:].rearrange("p (h c) -> p h c", h=HG),
            )

        def o_chunk(qh, qc):
            po = pr_tile(512, f"po{qh}{qc}")
            qs = slice(512 * qh + 128 * qc, 512 * qh + 128 * (qc + 1))
            for p2 in range(NPAIR):
                nc.tensor.matmul(
                    po[:],
                    z_t[p2][:, qs],
                    wo_t[p2][:],
                    start=(p2 == 0),
                    stop=(p2 == NPAIR - 1),
                )
            ot = out_pool.tile([128, D], F32, tag="ot")
            nc.vector.tensor_add(ot[:], po[:], bo_bc[:])
            nc.sync.dma_start(out[qs, :], ot[:])

        # ---- attention slot machinery ----
        p_slabs = {}
        z_tiles = [None]

        def s_slot(qh, pair, kb, sl):
            st = ps.tile([128, 1024], F32, tag=f"s{sl % 2}",
                         name=f"s{qh}_{pair}_{kb}", padded_shape=[128, 1024])
            qs = slice(512 * qh, 512 * (qh + 1))
            ks = slice(128 * kb, 128 * (kb + 1))
            nc.tensor.matmul(st[:, 0:512], k_t[pair][0:DH, ks],
                             q_t[pair][0:DH, qs],
                             start=True, stop=True, tile_position=(0, 0))
            nc.tensor.matmul(st[:, 512:1024], k_t[pair][DH:128, ks],
                             q_t[pair][DH:128, qs],
                             start=True, stop=True, tile_position=(64, 0))
            pab = p_pool.tile([128, 1024], BF16, tag="p",
                              name=f"p{qh}_{pair}_{kb}")
            nc.scalar.activation(pab[:], st[:], Exp, scale=0.125)
            if dbg and (qh, pair, kb) == (0, 0, 0):
                nc.sync.dma_start(dbg_out["P000"][:], pab[:])
            p_slabs[(qh, pair, kb)] = pab

        def z_alloc():
            zA = ps.tile([DH + 1, 512], F32, tag="zA", name="zA",
                         padded_shape=[128, 512])
            zB = ps.tile([DH + 1, 512], F32, tag="zB", name="zB",
                         padded_shape=[128, 512])
            z_tiles[0] = (zA, zB)

        def z_group(qh, pair, kb):
            if kb == 0:
                z_alloc()
            zA, zB = z_tiles[0]
            pab = p_slabs.pop((qh, pair, kb))
            hA, hB = 2 * pair, 2 * pair + 1
            nc.tensor.matmul(zA[:], v_aug[kb][:, 65 * hA:65 * hA + 65],
                             pab[:, 0:512],
                             start=(kb == 0), stop=(kb == NKB - 1),
                             skip_group_check=True)
            nc.tensor.matmul(zB[:], v_aug[kb][:, 65 * hB:65 * hB + 65],
                             pab[:, 512:1024],
                             start=(kb == 0), stop=(kb == NKB - 1),
                             skip_group_check=True)

        def norm(qh, pair):
            zA, zB = z_tiles[0]
            zcs = []
            for z_ps, half in ((zA, 0), (zB, 1)):
                # copy PSUM->SBUF first so both z banks free up fast
                zc = nrm_pool.tile([DH + 1, 512], F32, tag=f"zc{half}")
                nc.vector.tensor_copy(zc[:], z_ps[:])
                zcs.append(zc)
            for zc, half in zip(zcs, (0, 1)):
                # custom-DVE recip needs a fresh partition-0 source tile
                row = nrm_pool.tile([1, 512], F32, tag="row")
                nc.vector.tensor_copy(row[:], zc[DH:DH + 1, :])
                recip = nrm_pool.tile([1, 512], F32, tag="recip")
                nc.vector.reciprocal_approx_fast(recip[:], row[:])
                rbc = nrm_pool.tile([DH, 512], F32, tag="rbc")
                nc.gpsimd.partition_broadcast(rbc[:], recip[:], channels=DH)
                nc.vector.tensor_mul(
                    z_t[pair][64 * half:64 * half + 64,
                              512 * qh:512 * (qh + 1)],
                    zc[0:DH, :], rbc[:],
                )
                if dbg and (qh, pair) == (0, 0):
                    nc.sync.dma_start(dbg_out[f"ZC00{half}"][:], zc[:])
                    nc.sync.dma_start(dbg_out[f"RC00{half}"][:], recip[:])

        # ---- feeder schedule (ordered by first-use time with margin) ----
        def kq(dst, w_ts, b_all, pair, cb):
            return lambda: kq_chunk(dst, w_ts, b_all, pair, cb)

        def vch(kb):
            return lambda: v_chunk(kb)

        # ordered by first-use slot; group 0 consumes 2/slot in slots 0-3,
        # then 1/slot (see feed calls below)
        feeder = [
            kq(k_t, wk_t, bk_all, 0, 1), vch(0),
            kq(q_t, wq_t, bq_all, 1, 0), vch(1),
            kq(k_t, wk_t, bk_all, 0, 2), vch(2),
            kq(k_t, wk_t, bk_all, 0, 3), vch(3),
            vch(4), vch(5), vch(6), vch(7), vch(8), vch(9), vch(10),
            kq(k_t, wk_t, bk_all, 1, 0),
            vch(11), vch(12), vch(13), vch(14), vch(15),
            kq(k_t, wk_t, bk_all, 1, 1),
            kq(k_t, wk_t, bk_all, 1, 2),
            kq(k_t, wk_t, bk_all, 1, 3),
        ]
        for qh in range(1, NQH):
            for pair in range(NPAIR):
                feeder.append(kq(q_t, wq_t, bq_all, pair, qh))
        fi = [0]
        o_queue = []
        oi = [0]

        def feed(n):
            for _ in range(n):
                if fi[0] < len(feeder):
                    feeder[fi[0]]()
                    fi[0] += 1

        def feed_o(n):
            for _ in range(n):
                if oi[0] < len(o_queue):
                    o_queue[oi[0]]()
                    oi[0] += 1

        # ---- ramp: minimum to start slot 0 ----
        kq_chunk(k_t, wk_t, bk_all, 0, 0)
        kq_chunk(q_t, wq_t, bq_all, 0, 0)

        # ---- main loop: 8 groups x 16 slots ----
        groups = [(qh, pair) for qh in range(NQH) for pair in range(NPAIR)]
        Z_LAG = 4
        NG = len(groups)
        for gi, (qh, pair) in enumerate(groups):
            for t in range(NKB):
                s_slot(qh, pair, t, gi * NKB + t)
                if t >= Z_LAG:
                    z_group(qh, pair, t - Z_LAG)
                if gi > 0 and t < Z_LAG:
                    pqh, ppair = groups[gi - 1]
                    z_group(pqh, ppair, NKB - Z_LAG + t)
                    if t == Z_LAG - 1:
                        norm(pqh, ppair)
                        if ppair == NPAIR - 1:
                            # schedule output chunks for the finished q-block
                            for qc in range(4):
                                o_queue.append(
                                    (lambda a, b: (lambda: o_chunk(a, b)))
                                    (pqh, qc))
                # group 0 slots 0-3 are z-free: extra feed capacity there
                feed(2 if (gi == 0 and t < 4) else 1)
                # output chunks only after the norm chain has fully drained
                if t in (10, 12, 14):
                    feed_o(1)

        # ---- drain: last group's tail z + norm, with the final q-block's
        # output projection split by pair so pair-0 matmuls overlap the
        # last exp slots (pair-1 accumulates after the final norm) ----
        lqh, lpair = groups[-1]
        feed(len(feeder))
        feed_o(len(o_queue))
        po_drain = []
        for qc, tag in enumerate(("pr0", "pr1", "s0", "s1")):
            po = ps.tile([128, D], F32, tag=tag, name=f"pod{qc}",
                         padded_shape=[128, 512 if tag[0] == "p" else 1024])
            qs = slice(512 * lqh + 128 * qc, 512 * lqh + 128 * (qc + 1))
            nc.tensor.matmul(po[:], z_t[0][:, qs], wo_t[0][:],
                             start=True, stop=False, skip_group_check=True)
            po_drain.append(po)
            if qc == 1:
                for kb in range(NKB - Z_LAG, NKB - 2):
                    z_group(lqh, lpair, kb)
            elif qc == 2:
                z_group(lqh, lpair, NKB - 2)
        z_group(lqh, lpair, NKB - 1)
        norm(lqh, lpair)
        for qc in range(4):
            po = po_drain[qc]
            qs = slice(512 * lqh + 128 * qc, 512 * lqh + 128 * (qc + 1))
            nc.tensor.matmul(po[:], z_t[1][:, qs], wo_t[1][:],
                             start=False, stop=True, skip_group_check=True)
            ot = out_pool.tile([128, D], F32, tag="ot")
            nc.vector.tensor_add(ot[:], po[:], bo_bc[:])
            nc.sync.dma_start(out[qs, :], ot[:])

        if dbg:
            nc.sync.dma_start(dbg_out["KT0"][:], k_t[0][:])
            nc.sync.dma_start(dbg_out["QT0"][:], q_t[0][:])
            nc.sync.dma_start(dbg_out["VA0"][:], v_aug[0][:])
            nc.sync.dma_start(dbg_out["ZT0"][:], z_t[0][:])
            nc.sync.dma_start(dbg_out["ZT1"][:], z_t[1][:])

    nc.compile()
    return nc


_NC = None
LAST_RESULTS = None


def _get_nc():
    global _NC
    if _NC is None:
        _NC = build_program(dbg=bool(int(os.environ.get("KERNEL_DEBUG", "0"))))
    return _NC


def _bf(x):
    return np.ascontiguousarray(np.asarray(x).astype(ml_dtypes.bfloat16))


def kernel(Q, K, V, Wq, bq, Wk, bk, Wv, bv, Wo, bo):
    global LAST_RESULTS
    nc = _get_nc()
    Qb, Kb, Vb = _bf(Q), _bf(K), _bf(V)
    Wqb, Wkb, Wvb, Wob = _bf(Wq), _bf(Wk), _bf(Wv), _bf(Wo)
    bqf = np.asarray(bq, np.float32)
    bkf = np.asarray(bk, np.float32)
    bvf = np.asarray(bv, np.float32)
    bof = np.asarray(bo, np.float32)
    def tile128(a):
        # [R, C] -> SBUF tile layout [128, (R//128)*C]
        r, c = a.shape
        return np.ascontiguousarray(
            a.reshape(r // 128, 128, c).transpose(1, 0, 2).reshape(128, -1))

    QbT = Qb.transpose(0, 2, 1)
    KbT = Kb.transpose(0, 2, 1)
    VbT = Vb.transpose(0, 2, 1)
    in_maps = []
    for c in range(N_CORES):
        b, hg = c // 2, c % 2
        fs = slice(DG * hg, DG * (hg + 1))
        in_maps.append({
            "XQT": tile128(QbT[b]),
            "XKT": tile128(KbT[b]),
            "XVT": tile128(VbT[b]),
            "Wq": tile128(Wqb[:, fs]),
            "Wk": tile128(Wkb[:, fs]),
            "Wv": tile128(Wvb[:, fs]),
            "Wo": tile128(Wob[fs, :]),
            "bq": np.ascontiguousarray(bqf[fs].reshape(DG, 1)),
            "bk": np.ascontiguousarray(bkf[fs].reshape(DG, 1)),
            "bv": np.ascontiguousarray(bvf[fs].reshape(1, DG)),
            "bo": np.ascontiguousarray((bof * 0.5).reshape(1, D)),
        })
    trace = bool(int(os.environ.get("KERNEL_TRACE", "0")))
    res = bass_utils.run_bass_kernel_spmd(
        nc, in_maps, core_ids=list(range(N_CORES)), trace=trace,
    )
    LAST_RESULTS = res
    out = np.empty((B, S, D), dtype=np.float32)
    for b in range(B):
        out[b] = res.results[2 * b]["OUT"] + res.results[2 * b + 1]["OUT"]
    return out
